# revision 3
# baseline (speedup 1.0000x reference)
"""DelayLineRNN2xOS Trainium2 Bass kernel.

Problem (per reference):
  B=64, T=1024, D=1024, H=1024
  gi = x @ W_ih.T + b_ih + b_hh
  two independent tanh-RNN chains over T/2 steps:
    eve chain consumes gi at times 0,2,...,T-2
    odd chain consumes gi at times T-1,1,3,...,T-3
  states[:, 2i] = h_eve(i); states[:, (2i-1) % T] = h_odd(i)
  returns (states [B,T,H], h_eve_final [1,B,H])

Strategy: data-parallel over batch across 8 cores (8 batches/core).
Per core, the two chains x 8 batches = 16 "streams" advance in lockstep.

Layouts (per core, all fp16 on device, fp32 PSUM accumulation):
  state h kept as [128 part = h%128, 16*m + j] (m = h//128 tile, j = stream)
  recurrence m-tile matmul: psum[:, 16m+j] = sum_k W_hhT[k-tile][:, m-slice].T @ h[:, k-slice]
  g folded into PSUM via identity matmul: lhsT = gT[16 streams, h-slice], rhs = I16
  projection computes gT directly ([s'-tile partition, h free]) by making
  x^T tiles the stationary operand; s' is host-permuted so s'-tiles are
  i-blocks per (parity): no on-device transposes anywhere.
"""

import numpy as np

import concourse.bacc as bacc
import concourse.mybir as mybir
import concourse.tile as tile
from concourse.bass_utils import run_bass_kernel_spmd
from concourse.masks import make_identity

F32 = mybir.dt.float32
F16 = mybir.dt.float16
TANH = mybir.ActivationFunctionType.Tanh

NCORES = 8
BS = 8  # batches per core
J = 2 * BS  # streams (eve batches then odd batches)


def build(T=1024, D=1024, H=1024, out_group=4):
    """Build the per-core Bass program. Shapes hardcoded for the SPMD shard."""
    I = T // 2  # sequential iterations
    KT = D // 128  # contraction tiles (projection)
    MT = H // 128  # h tiles
    HT = T // 256  # i-blocks of 128 per parity
    ST = T // 128  # s' tiles
    HC = (H + 511) // 512  # h chunks of <=512 for projection psum
    HCS = min(H, 512)
    assert T % 256 == 0 and D % 128 == 0 and H % 128 == 0 and H % HCS == 0
    assert MT % 2 == 0 and I % out_group == 0

    nc = bacc.Bacc("TRN2", target_bir_lowering=False, debug=False)

    xT = nc.dram_tensor("xT", (BS, D, T), F16, kind="ExternalInput").ap()
    wihT = nc.dram_tensor("wihT", (D, H), F16, kind="ExternalInput").ap()
    whhT = nc.dram_tensor("whhT", (H, H), F16, kind="ExternalInput").ap()
    bias = nc.dram_tensor("bias", (H,), F32, kind="ExternalInput").ap()
    out_raw = nc.dram_tensor("out_raw", (128, I, MT * 16), F16, kind="ExternalOutput").ap()

    with tile.TileContext(nc) as tc:
        with (
            tc.tile_pool(name="wpool", bufs=1) as wpool,
            tc.tile_pool(name="dram", bufs=1, space="DRAM") as dram,
            tc.tile_pool(name="xpool", bufs=16) as xpool,
            tc.tile_pool(name="ppool", bufs=2 * HC, space="PSUM") as ppool,
            tc.tile_pool(name="gspool", bufs=3) as gspool,
            tc.tile_pool(name="rpool", bufs=1, space="PSUM") as rpool,
            tc.tile_pool(name="gpool", bufs=4) as gpool,
            tc.tile_pool(name="hpool", bufs=2) as hpool,
        ):
            # ---- persistent setup ----
            whh_sb = wpool.tile([128, MT, H], F16)  # [k%128, k//128, h_out]
            nc.sync.dma_start(whh_sb[:], whhT.rearrange("(k p) m -> p k m", p=128))
            wih_sb = wpool.tile([128, KT, H], F16)
            nc.sync.dma_start(wih_sb[:], wihT.rearrange("(k p) m -> p k m", p=128))
            bias_rep = wpool.tile([128, H], F32)
            nc.sync.dma_start(bias_rep[:], bias[None, :].to_broadcast((128, H)))
            i16 = wpool.tile([16, 16], F16)
            make_identity(nc, i16[:])
            h0 = wpool.tile([128, MT * 16], F16)
            nc.gpsimd.memset(h0[:], 0.0)

            gT_dev = dram.tile([J, I, H], F16)  # [stream, iter, h]

            # ---- projection phase: gT_dev[j, i, :] = x[b_j, t_j(i), :] @ W_ih.T + bias
            # s'-tile order interleaves parities so early iters complete first.
            st_order = [st for pair in zip(range(HT), range(HT, ST)) for st in pair]
            for st in st_order:
                for b in range(BS):
                    j = b + BS * (st // HT)
                    ib = (st % HT) * 128
                    xts = []
                    for dt in range(KT):
                        xt = xpool.tile([128, 128], F16, tag="xt")
                        nc.sync.dma_start(
                            xt[:],
                            xT[b, dt * 128 : (dt + 1) * 128, st * 128 : (st + 1) * 128],
                        )
                        xts.append(xt)
                    pss = [
                        ppool.tile([128, HCS], F32, tag="pp", name=f"pp{hc}")
                        for hc in range(HC)
                    ]
                    for dt in range(KT):
                        for hc in range(HC):
                            nc.tensor.matmul(
                                pss[hc][:],
                                xts[dt][:],
                                wih_sb[:, dt, hc * HCS : (hc + 1) * HCS],
                                start=(dt == 0),
                                stop=(dt == KT - 1),
                            )
                    for hc in range(HC):
                        gsb = gspool.tile([128, HCS], F16, tag="gsb")
                        nc.vector.tensor_tensor(
                            gsb[:],
                            pss[hc][:],
                            bias_rep[:, hc * HCS : (hc + 1) * HCS],
                            mybir.AluOpType.add,
                        )
                        nc.sync.dma_start(
                            gT_dev[j, ib : ib + 128, hc * HCS : (hc + 1) * HCS], gsb[:]
                        )

            # ---- recurrence ----
            h_prev = h0
            out4 = None
            for i in range(I):
                g_sb = gpool.tile([J, H], F16, tag="g")
                nc.sync.dma_start(g_sb[:], gT_dev[:, i, :])
                if i % out_group == 0:
                    out4 = hpool.tile([128, out_group * MT * 16], F16, tag="out4")
                h_new = out4[
                    :, (i % out_group) * MT * 16 : (i % out_group + 1) * MT * 16
                ]
                for t in range(MT // 2):
                    ps = rpool.tile([128, 32], F32, tag=f"rp{t % 4}")
                    for q in range(2):
                        m = 2 * t + q
                        sl = ps[:, 16 * q : 16 * q + 16]
                        nc.tensor.matmul(
                            sl,
                            g_sb[:, m * 128 : (m + 1) * 128],
                            i16[:],
                            start=True,
                            stop=False,
                        )
                        for k in range(MT):
                            nc.tensor.matmul(
                                sl,
                                whh_sb[:, k, m * 128 : (m + 1) * 128],
                                h_prev[:, 16 * k : 16 * k + 16],
                                start=False,
                                stop=(k == MT - 1),
                            )
                    nc.scalar.activation(
                        h_new[:, 32 * t : 32 * t + 32], ps[:], TANH
                    )
                h_prev = h_new
                if i % out_group == out_group - 1:
                    nc.sync.dma_start(
                        out_raw[:, i - out_group + 1 : i + 1, :].rearrange(
                            "p i c -> p (i c)"
                        ),
                        out4[:],
                    )

    nc.compile()
    return nc


def host_prep(x, W_ih, W_hh, b_ih, b_hh):
    """Host-side permute/transpose/cast. Returns per-core in_maps."""
    B, T, D = x.shape
    idx = np.empty(T, np.int64)
    idx[: T // 2] = 2 * np.arange(T // 2)
    idx[T // 2 :] = (2 * np.arange(T // 2) - 1) % T
    xp = x[:, idx, :]  # [B, s', D]
    xT = np.ascontiguousarray(xp.transpose(0, 2, 1)).astype(np.float16)  # [B, D, s']
    wihT = np.ascontiguousarray(np.asarray(W_ih).T).astype(np.float16)
    whhT = np.ascontiguousarray(np.asarray(W_hh).T).astype(np.float16)
    biasv = (np.asarray(b_ih) + np.asarray(b_hh)).astype(np.float32)
    ncores = B // BS
    return [
        {
            "xT": xT[BS * c : BS * (c + 1)],
            "wihT": wihT,
            "whhT": whhT,
            "bias": biasv,
        }
        for c in range(ncores)
    ]


def assemble(outs, B, T, H):
    """outs: per-core out_raw [128, T//2, (H//128)*16] fp16 -> (states, h_final)."""
    I = T // 2
    MT = H // 128
    states = np.empty((B, T, H), np.float32)
    for c, v in enumerate(outs):
        v = v.astype(np.float32).reshape(128, I, MT, 16)  # [p, i, m, j]
        v = v.transpose(3, 1, 2, 0).reshape(J, I, H)  # [j, i, (m p)=h]
        eve, odd = v[:BS], v[BS:]
        states[BS * c : BS * (c + 1), 0::2] = eve
        states[BS * c : BS * (c + 1), 1::2] = np.roll(odd, -1, axis=1)
    h_final = states[:, T - 2].copy()[None]
    return states, h_final


_NC_CACHE = {}


def kernel(x, W_ih, W_hh, b_ih, b_hh):
    x = np.asarray(x)
    B, T, D = x.shape
    H = np.asarray(W_hh).shape[0]
    in_maps = host_prep(x, W_ih, W_hh, b_ih, b_hh)
    key = (T, D, H)
    if key not in _NC_CACHE:
        _NC_CACHE[key] = build(T=T, D=D, H=H)
    nc = _NC_CACHE[key]
    res = run_bass_kernel_spmd(nc, in_maps, core_ids=list(range(len(in_maps))))
    outs = [r["out_raw"] for r in res.results]
    return assemble(outs, B, T, H)


# revision 8
# speedup vs baseline: 320.0091x; 320.0091x over previous
"""DelayLineRNN2xOS Trainium2 Bass kernel.

Problem (per reference):
  B=64, T=1024, D=1024, H=1024
  gi = x @ W_ih.T + b_ih + b_hh
  two independent tanh-RNN chains over T/2 steps:
    eve chain consumes gi at times 0,2,...,T-2
    odd chain consumes gi at times T-1,1,3,...,T-3
  states[:, 2i] = h_eve(i); states[:, (2i-1) % T] = h_odd(i)
  returns (states [B,T,H], h_eve_final [1,B,H])

Strategy: data-parallel over batch across 8 cores (8 batches/core).
Per core, the two chains x 8 batches = 16 "streams" advance in lockstep.

Layouts (per core, all fp16 on device, fp32 PSUM accumulation):
  state h kept as [128 part = h%128, 16*m + j] (m = h//128 tile, j = stream)
  recurrence m-tile matmul: psum[:, 16m+j] = sum_k W_hhT[k-tile][:, m-slice].T @ h[:, k-slice]
  g folded into PSUM via identity matmul: lhsT = gT[16 streams, h-slice], rhs = I16
  projection computes gT directly ([s'-tile partition, h free]) by making
  x^T tiles the stationary operand; s' is host-permuted so s'-tiles are
  i-blocks per (parity): no on-device transposes anywhere.
"""

import numpy as np

import concourse.bacc as bacc
import concourse.mybir as mybir
import concourse.tile as tile
from concourse.bass_utils import run_bass_kernel_spmd
from concourse.masks import make_identity

F32 = mybir.dt.float32
F16 = mybir.dt.float16
TANH = mybir.ActivationFunctionType.Tanh

NCORES = 8
BS = 8  # batches per core
J = 2 * BS  # streams (eve batches then odd batches)


def build(T=1024, D=1024, H=1024, out_group=4, rec_iters=None):
    """Build the per-core Bass program. Shapes hardcoded for the SPMD shard.

    rec_iters: if set, truncate the recurrence to this many iterations
    (timing experiments only — output is then incomplete).
    """
    I = T // 2  # sequential iterations
    KT = D // 128  # contraction tiles (projection)
    MT = H // 128  # h tiles
    HT = T // 256  # i-blocks of 128 per parity
    ST = T // 128  # s' tiles
    HC = (H + 511) // 512  # h chunks of <=512 for projection psum
    HCS = min(H, 512)
    assert T % 256 == 0 and D % 128 == 0 and H % 128 == 0 and H % HCS == 0
    assert MT % 2 == 0 and I % out_group == 0

    nc = bacc.Bacc("TRN2", target_bir_lowering=False, debug=False)

    xT = nc.dram_tensor("xT", (BS, D, T), F16, kind="ExternalInput").ap()
    wihT = nc.dram_tensor("wihT", (D, H), F16, kind="ExternalInput").ap()
    whhT = nc.dram_tensor("whhT", (H, H), F16, kind="ExternalInput").ap()
    bias = nc.dram_tensor("bias", (H,), F32, kind="ExternalInput").ap()
    out_raw = nc.dram_tensor("out_raw", (128, I, MT * 16), F16, kind="ExternalOutput").ap()

    with tile.TileContext(nc) as tc:
        with (
            tc.tile_pool(name="wpool", bufs=1) as wpool,
            tc.tile_pool(name="dram", bufs=1, space="DRAM") as dram,
            tc.tile_pool(name="xpool", bufs=16) as xpool,
            tc.tile_pool(name="ppool", bufs=2 * HC, space="PSUM") as ppool,
            tc.tile_pool(name="gspool", bufs=3) as gspool,
            tc.tile_pool(name="rpool", bufs=2, space="PSUM") as rpool,
            tc.tile_pool(name="hpool", bufs=2) as hpool,
        ):
            # ---- persistent setup ----
            whh_sb = wpool.tile([128, MT, H], F16)  # [k%128, k//128, h_out]
            nc.sync.dma_start(whh_sb[:], whhT.rearrange("(k p) m -> p k m", p=128))
            wih_sb = wpool.tile([128, KT, H], F16)
            nc.sync.dma_start(wih_sb[:], wihT.rearrange("(k p) m -> p k m", p=128))
            bias_rep = wpool.tile([128, H], F32)
            nc.sync.dma_start(bias_rep[:], bias[None, :].to_broadcast((128, H)))
            # identity [128,16]: I16 on top, zeros below (kills padded-K garbage)
            i16 = wpool.tile([128, 16], F16)
            nc.gpsimd.memset(i16[:], 0.0)
            make_identity(nc, i16[:16, :], nomemset=True)
            h0 = wpool.tile([128, MT * 16], F16)
            nc.gpsimd.memset(h0[:], 0.0)
            # persistent ring of K-padded g tiles (partitions 16.. stay zero)
            GRING = 4
            grings = []
            for r in range(GRING):
                gr = wpool.tile([128, H], F16, name=f"gring{r}")
                nc.gpsimd.memset(gr[:], 0.0)
                grings.append(gr)

            gT_dev = dram.tile([J, I, H], F16)  # [stream, iter, h]

            # ---- projection phase: gT_dev[j, i, :] = x[b_j, t_j(i), :] @ W_ih.T + bias
            # s'-tile order interleaves parities so early iters complete first.
            st_order = [st for pair in zip(range(HT), range(HT, ST)) for st in pair]
            for st in st_order:
                for b in range(BS):
                    j = b + BS * (st // HT)
                    ib = (st % HT) * 128
                    xts = []
                    for dt in range(KT):
                        xt = xpool.tile([128, 128], F16, tag="xt")
                        nc.sync.dma_start(
                            xt[:],
                            xT[b, dt * 128 : (dt + 1) * 128, st * 128 : (st + 1) * 128],
                        )
                        xts.append(xt)
                    pss = [
                        ppool.tile([128, HCS], F32, tag="pp", name=f"pp{hc}")
                        for hc in range(HC)
                    ]
                    for dt in range(KT):
                        for hc in range(HC):
                            nc.tensor.matmul(
                                pss[hc][:],
                                xts[dt][:],
                                wih_sb[:, dt, hc * HCS : (hc + 1) * HCS],
                                start=(dt == 0),
                                stop=(dt == KT - 1),
                            )
                    for hc in range(HC):
                        gsb = gspool.tile([128, HCS], F16, tag="gsb")
                        nc.vector.tensor_tensor(
                            gsb[:],
                            pss[hc][:],
                            bias_rep[:, hc * HCS : (hc + 1) * HCS],
                            mybir.AluOpType.add,
                        )
                        nc.sync.dma_start(
                            gT_dev[j, ib : ib + 128, hc * HCS : (hc + 1) * HCS], gsb[:]
                        )

            # ---- recurrence ----
            h_prev = h0
            out4 = None
            GPM = 4  # m-groups per psum tile
            for i in range(I if rec_iters is None else rec_iters):
                g_sb = grings[i % GRING]
                nc.sync.dma_start(g_sb[:J, :], gT_dev[:, i, :])
                if i % out_group == 0:
                    out4 = hpool.tile([128, out_group * MT * 16], F16, tag="out4")
                h_new = out4[
                    :, (i % out_group) * MT * 16 : (i % out_group + 1) * MT * 16
                ]
                for t in range((MT + GPM - 1) // GPM):
                    ngrp = min(GPM, MT - t * GPM)
                    ps = rpool.tile([128, 16 * GPM], F32, tag=f"rp{t % 2}")
                    for q in range(ngrp):
                        m = GPM * t + q
                        sl = ps[:, 16 * q : 16 * q + 16]
                        nc.tensor.matmul(
                            sl,
                            g_sb[:, m * 128 : (m + 1) * 128],
                            i16[:],
                            start=True,
                            stop=False,
                        )
                        for k in range(MT):
                            nc.tensor.matmul(
                                sl,
                                whh_sb[:, k, m * 128 : (m + 1) * 128],
                                h_prev[:, 16 * k : 16 * k + 16],
                                start=False,
                                stop=(k == MT - 1),
                            )
                    nc.scalar.activation(
                        h_new[:, 16 * GPM * t : 16 * GPM * t + 16 * ngrp],
                        ps[:, : 16 * ngrp],
                        TANH,
                    )
                h_prev = h_new
                if i % out_group == out_group - 1:
                    nc.sync.dma_start(
                        out_raw[:, i - out_group + 1 : i + 1, :].rearrange(
                            "p i c -> p (i c)"
                        ),
                        out4[:],
                    )

    nc.compile()
    return nc


def host_prep(x, W_ih, W_hh, b_ih, b_hh):
    """Host-side permute/transpose/cast. Returns per-core in_maps."""
    B, T, D = x.shape
    idx = np.empty(T, np.int64)
    idx[: T // 2] = 2 * np.arange(T // 2)
    idx[T // 2 :] = (2 * np.arange(T // 2) - 1) % T
    xp = x[:, idx, :]  # [B, s', D]
    xT = np.ascontiguousarray(xp.transpose(0, 2, 1)).astype(np.float16)  # [B, D, s']
    wihT = np.ascontiguousarray(np.asarray(W_ih).T).astype(np.float16)
    whhT = np.ascontiguousarray(np.asarray(W_hh).T).astype(np.float16)
    biasv = (np.asarray(b_ih) + np.asarray(b_hh)).astype(np.float32)
    ncores = B // BS
    return [
        {
            "xT": xT[BS * c : BS * (c + 1)],
            "wihT": wihT,
            "whhT": whhT,
            "bias": biasv,
        }
        for c in range(ncores)
    ]


def assemble(outs, B, T, H):
    """outs: per-core out_raw [128, T//2, (H//128)*16] fp16 -> (states, h_final)."""
    I = T // 2
    MT = H // 128
    states = np.empty((B, T, H), np.float32)
    for c, v in enumerate(outs):
        v = v.astype(np.float32).reshape(128, I, MT, 16)  # [p, i, m, j]
        v = v.transpose(3, 1, 2, 0).reshape(J, I, H)  # [j, i, (m p)=h]
        eve, odd = v[:BS], v[BS:]
        states[BS * c : BS * (c + 1), 0::2] = eve
        states[BS * c : BS * (c + 1), 1::2] = np.roll(odd, -1, axis=1)
    h_final = states[:, T - 2].copy()[None]
    return states, h_final


_NC_CACHE = {}


def kernel(x, W_ih, W_hh, b_ih, b_hh):
    x = np.asarray(x)
    B, T, D = x.shape
    H = np.asarray(W_hh).shape[0]
    in_maps = host_prep(x, W_ih, W_hh, b_ih, b_hh)
    key = (T, D, H)
    if key not in _NC_CACHE:
        _NC_CACHE[key] = build(T=T, D=D, H=H)
    nc = _NC_CACHE[key]
    res = run_bass_kernel_spmd(nc, in_maps, core_ids=list(range(len(in_maps))))
    outs = [r["out_raw"] for r in res.results]
    return assemble(outs, B, T, H)
